# revision 1
# baseline (speedup 1.0000x reference)
"""Trainium2 Bass kernel for nn_AFiReLoss (SwAV-style sinkhorn CE + recon MSE).

Distribution: data-parallel over batch B=64 across 8 NeuronCores (8 per core).
The teacher batch-sum is ReduceScattered so each core owns a K-shard of the
prototype; sinkhorn runs K-sharded (col sums all-reduced, 196 floats/iter);
the normalized teacher targets are AllGathered for the local CE pass.

Math notes:
  per_patch[b,l] = lse(student[b,l,:]/T) * rowsum_t[l] - (1/T) * <student, t>
  and after the final col-normalization of sinkhorn, rowsum_t[l] == 1 exactly.
  The initial Q/sum(Q) normalization cancels in the first row-normalization
  and is skipped (values stay well inside f32 range).
"""

import numpy as np
import ml_dtypes

import concourse.bass as bass
import concourse.mybir as mybir
from concourse import tile, bacc
from concourse.bass_utils import run_bass_kernel_spmd
from concourse.masks import make_identity

F32 = mybir.dt.float32
BF16 = mybir.dt.bfloat16
AX = mybir.AxisListType
ALU = mybir.AluOpType
AF = mybir.ActivationFunctionType

P = 128              # SBUF partitions
N_CORES = 8
STUDENT_TEMP = 0.1
PROTO_MOMENTUM = 0.75
SK_EPS = 0.05
SK_ITERS = 3
LSE_SHIFT = 25.0   # global exp shift; added back in finalize()


def _ceil_div(a, b):
    return (a + b - 1) // b


def _tree_reduce_sum(nc, t_ap_fn, width, out_ap):
    """Free-axis sum via in-place halving adds (bf16 TT runs 2x; reduce is
    1x-only), then one short reduce_sum. t_ap_fn(lo, hi) -> AP slice."""
    w = width
    while w >= 1024 and w % 2 == 0:
        h = w // 2
        nc.vector.tensor_add(t_ap_fn(0, h), t_ap_fn(0, h), t_ap_fn(h, w))
        w = h
    nc.vector.reduce_sum(out_ap, t_ap_fn(0, w), axis=AX.X)


def build_nc(B_loc=8, L=196, K=8192, C=N_CORES, R=9408, dummy_out=True,
             stop_after="full"):
    """Build the per-core SPMD graph. R = recon elements per partition.

    stop_after: "rs" | "sinkhorn" | "full" — debugging aid that truncates
    the graph after the named phase (outputs are then meaningless).
    """
    KSH = K // C
    n_lt = _ceil_div(L, P)
    lt_sizes = [min(P, L - i * P) for i in range(n_lt)]
    n_t = B_loc * n_lt               # CE tiles per core

    nc = bacc.Bacc("TRN2", target_bir_lowering=False, debug=False,
                   num_devices=C)

    student = nc.declare_dram_parameter("student", [B_loc, L, K], BF16, isOutput=False)
    teacher = nc.declare_dram_parameter("teacher", [B_loc, L, K], BF16, isOutput=False)
    recon = nc.declare_dram_parameter("recon", [P, R], BF16, isOutput=False)
    label = nc.declare_dram_parameter("label", [P, R], BF16, isOutput=False)
    proto = nc.declare_dram_parameter("proto", [L, KSH], F32, isOutput=False)
    maskp = nc.declare_dram_parameter("maskp", [P, n_t], F32, isOutput=False)
    cfg = nc.declare_dram_parameter("cfg", [P, 1], F32, isOutput=False)
    out_ext = nc.declare_dram_parameter("out", [2 * P, 1], F32, isOutput=True)

    groups = [list(range(C))]

    with tile.TileContext(nc) as tc:
        with (
            tc.tile_pool(name="dram", bufs=1, space="DRAM") as dram,
            tc.tile_pool(name="consts", bufs=1) as consts,
            tc.tile_pool(name="small", bufs=2) as small,
            tc.tile_pool(name="spool", bufs=2) as spool,
            tc.tile_pool(name="scrp", bufs=1) as scrp,
            tc.tile_pool(name="tpool", bufs=2) as tpool,
        ):
            # ---- DRAM scratch (dep-tracked via DRAM pool) ----
            bsum_tiled = dram.tile([C, L, KSH], F32, tag="bsum_tiled")
            bsum_shard = dram.tile([L, KSH], F32, tag="bsum_shard")
            t_shard = dram.tile([L, KSH], BF16, tag="t_shard")
            t_all = dram.tile([C, L, KSH], BF16, tag="t_all")
            col_io = [
                (dram.tile([L, 1], F32, tag=f"col_in{i}", name=f"col_in{i}"),
                 dram.tile([L, 1], F32, tag=f"col_out{i}", name=f"col_out{i}"))
                for i in range(SK_ITERS)
            ]

            # ---- constants ----
            idbf = consts.tile([P, P], BF16, tag="idbf")
            make_identity(nc, idbf[:, :])
            idf32 = consts.tile([P, P], F32, tag="idf32")
            make_identity(nc, idf32[:, :])
            ones1 = consts.tile([1, P], F32, tag="ones1")
            nc.gpsimd.memset(ones1[:, :], 1.0)
            cL = consts.tile([P, 1], F32, tag="cL")
            nc.gpsimd.memset(cL[:, :], float(L))
            cfg_sb = consts.tile([P, 1], F32, tag="cfg_sb")
            nc.sync.dma_start(cfg_sb[:, :], cfg[:, :])
            mask_sb = consts.tile([P, n_t], F32, tag="mask_sb")
            nc.sync.dma_start(mask_sb[:, :], maskp[:, :])

            res = consts.tile([P, n_t], F32, tag="res")
            nc.gpsimd.memset(res[:, :], 0.0)
            nshift = consts.tile([P, 1], F32, tag="nshift")
            nc.gpsimd.memset(nshift[:, :], -LSE_SHIFT)
            # dead-store sinks for fused-reduce ops (free-stride-0 writes)
            dummy_bf = consts.tile([P, 1], BF16, tag="dummy_bf")

            # =========================================================
            # Phase 1: teacher batch-sum via PE identity-matmul accum
            # =========================================================
            KH = 2048 if K >= 2048 else K      # K columns per PSUM round
            n_rounds = _ceil_div(K, KH)
            assert KH % KSH == 0 or KSH % KH == 0
            with (
                tc.tile_pool(name="teach", bufs=2) as teach,
                tc.tile_pool(name="bsp", bufs=4, space="PSUM") as bsp,
                tc.tile_pool(name="bsev", bufs=2) as bsev,
            ):
                for lt, nl in enumerate(lt_sizes):
                    l0 = lt * P
                    for r in range(n_rounds):
                        k0 = r * KH
                        kw = min(KH, K - k0)
                        n_ch = _ceil_div(kw, 512)
                        psums = [bsp.tile([P, 512], F32, tag="bs", name=f"bs{r}_{i}")
                                 for i in range(n_ch)]
                        for b in range(B_loc):
                            tt = teach.tile([P, KH], BF16, tag="tt")
                            nc.sync.dma_start(
                                tt[:nl, :kw], teacher[b, l0:l0 + nl, k0:k0 + kw])
                            for ch in range(n_ch):
                                c0 = ch * 512
                                cw = min(512, kw - c0)
                                nc.tensor.matmul(
                                    psums[ch][:nl, :cw],
                                    idbf[:nl, :nl],
                                    tt[:nl, c0:c0 + cw],
                                    start=(b == 0), stop=(b == B_loc - 1))
                        stage = bsev.tile([P, KH], F32, tag="bsum_stage",
                                          name=f"stage{lt}_{r}")
                        for ch in range(n_ch):
                            c0 = ch * 512
                            cw = min(512, kw - c0)
                            nc.scalar.copy(stage[:nl, c0:c0 + cw],
                                           psums[ch][:nl, :cw])
                        # write k-tiled layout: RS chunk c = core c's K-shard
                        cs0 = k0 // KSH
                        ncs = max(1, kw // KSH)
                        nc.sync.dma_start(
                            bsum_tiled[cs0:cs0 + ncs, l0:l0 + nl,
                                       (k0 % KSH):(k0 % KSH) + min(kw, KSH)]
                            .rearrange("c l k -> l c k"),
                            stage[:nl, :kw].rearrange("l (c k) -> l c k", c=ncs))

            # =========================================================
            # Phase 1b: recon MSE partial on DVE
            # =========================================================
            RCH = 2352 if R > 2352 else R
            n_rch = _ceil_div(R, RCH)
            with tc.tile_pool(name="rec", bufs=1) as rec:
                rsq_prev = None
                for rc in range(n_rch):
                    r0 = rc * RCH
                    rw = min(RCH, R - r0)
                    r_sb = rec.tile([P, RCH], BF16, tag="r_sb", name=f"r_sb{rc}")
                    l_sb = rec.tile([P, RCH], BF16, tag="l_sb", name=f"l_sb{rc}")
                    d_sb = rec.tile([P, RCH], BF16, tag="d_sb", name=f"d_sb{rc}")
                    nc.sync.dma_start(r_sb[:, :rw], recon[:, r0:r0 + rw])
                    nc.sync.dma_start(l_sb[:, :rw], label[:, r0:r0 + rw])
                    rsq = small.tile([P, 1], F32, tag="rsq", name=f"rsq{rc}")
                    nc.vector.tensor_sub(d_sb[:, :rw], r_sb[:, :rw], l_sb[:, :rw])
                    nc.vector.tensor_mul(d_sb[:, :rw], d_sb[:, :rw], d_sb[:, :rw])
                    nc.vector.reduce_sum(rsq[:, 0:1], d_sb[:, :rw], axis=AX.X)
                    if rsq_prev is not None:
                        rsq2 = small.tile([P, 1], F32, tag="rsqs",
                                          name=f"rsqs{rc}")
                        nc.vector.tensor_add(rsq2[:, 0:1], rsq[:, 0:1],
                                             rsq_prev[:, 0:1])
                        rsq = rsq2
                    rsq_prev = rsq

            # =========================================================
            # Phase 2: ReduceScatter the batch-sum -> own K-shard
            # =========================================================
            nc.gpsimd.collective_compute(
                "ReduceScatter", ALU.add, replica_groups=groups,
                ins=[bsum_tiled.opt()], outs=[bsum_shard.opt()])

            if stop_after != "rs":
                # =========================================================
                # Phase 3: K-sharded sinkhorn on [L, KSH] f32
                # =========================================================
                n_kch = _ceil_div(KSH, P)          # 128-wide chunks of the shard
                with (
                    tc.tile_pool(name="skp", bufs=1) as skp,
                    tc.tile_pool(name="skpp", bufs=2, space="PSUM") as skpp,
                ):
                    Q = []
                    for lt, nl in enumerate(lt_sizes):
                        l0 = lt * P
                        pr = skp.tile([P, KSH], F32, tag=f"pr{lt}")
                        sh = skp.tile([P, KSH], F32, tag=f"sh{lt}")
                        nc.sync.dma_start(pr[:nl, :], proto[l0:l0 + nl, :])
                        nc.sync.dma_start(sh[:nl, :], bsum_shard[l0:l0 + nl, :])
                        q = skp.tile([P, KSH], F32, tag=f"q{lt}")
                        # q = exp(sh * cfg + pr)   (cfg = (1-m)/(64*eps) per-partition)
                        nc.vector.scalar_tensor_tensor(
                            q[:nl, :], in0=sh[:nl, :], scalar=cfg_sb[:nl, 0:1],
                            in1=pr[:nl, :], op0=ALU.mult, op1=ALU.add)
                        nc.scalar.activation(q[:nl, :], q[:nl, :], AF.Exp)
                        Q.append(q)

                    rb = skp.tile([P, KSH], F32, tag="rb")
                    for it in range(SK_ITERS):
                        # --- row step: r[k] = 1/(L * rowsum[k]), rowsum over l ---
                        ps_r = skpp.tile([P, n_kch], F32, tag="ps_r")
                        for ch in range(n_kch):
                            c0 = ch * P
                            cw = min(P, KSH - c0)
                            for lt, nl in enumerate(lt_sizes):
                                nc.tensor.matmul(
                                    ps_r[:cw, ch:ch + 1],
                                    Q[lt][:nl, c0:c0 + cw],
                                    cL[:nl, 0:1],
                                    start=(lt == 0), stop=(lt == n_lt - 1))
                        rmax = min(P, KSH)
                        rrec = small.tile([P, n_kch], F32, tag="rrec")
                        if rmax < P:
                            nc.vector.memset(rrec[:, :], 1.0)
                        nc.vector.reciprocal(rrec[:rmax, :], ps_r[:rmax, :])
                        # per chunk: transpose rowrecip column to a partition-0
                        # row, then PE-broadcast it across all 128 partitions
                        # (matmul operands must start at partition 0/32/64).
                        ps_b = skpp.tile([P, KSH], F32, tag="ps_b")
                        for ch in range(n_kch):
                            c0 = ch * P
                            cw = min(P, KSH - c0)
                            ps_t = skpp.tile([1, P], F32, tag="ps_t",
                                             name=f"ps_t{it}_{ch}")
                            nc.tensor.transpose(ps_t[0:1, :], rrec[:, ch:ch + 1],
                                                idf32[:, :])
                            r8row = small.tile([1, P], F32, tag="r8row",
                                               name=f"r8row{it}_{ch}")
                            nc.scalar.copy(r8row[0:1, :], ps_t[0:1, :])
                            nc.tensor.matmul(
                                ps_b[:, c0:c0 + cw],
                                ones1[0:1, :],
                                r8row[0:1, :cw],
                                start=True, stop=True)
                        nc.scalar.copy(rb[:, :], ps_b[:, :])
                        for lt, nl in enumerate(lt_sizes):
                            nc.vector.tensor_mul(Q[lt][:nl, :], Q[lt][:nl, :], rb[:nl, :])

                        # --- col step: c[l] = 1/(K * colsum[l]), colsum all-reduced ---
                        col_in, col_out = col_io[it]
                        colp = small.tile([P, 1], F32, tag="colp")
                        for lt, nl in enumerate(lt_sizes):
                            l0 = lt * P
                            nc.vector.reduce_sum(colp[:nl, 0:1], Q[lt][:nl, :], axis=AX.X)
                            nc.sync.dma_start(col_in[l0:l0 + nl, 0:1], colp[:nl, 0:1])
                        nc.gpsimd.collective_compute(
                            "AllReduce", ALU.add, replica_groups=groups,
                            ins=[col_in.opt()], outs=[col_out.opt()])
                        for lt, nl in enumerate(lt_sizes):
                            l0 = lt * P
                            csb = small.tile([P, 1], F32, tag="csb")
                            nc.sync.dma_start(csb[:nl, 0:1], col_out[l0:l0 + nl, 0:1])
                            nc.vector.tensor_scalar_mul(csb[:nl, 0:1], csb[:nl, 0:1],
                                                        float(K))
                            crec = small.tile([P, 1], F32, tag="crec")
                            nc.vector.reciprocal(crec[:nl, 0:1], csb[:nl, 0:1])
                            nc.vector.tensor_scalar_mul(Q[lt][:nl, :], Q[lt][:nl, :],
                                                        crec[:nl, 0:1])

                    # final targets: t = Q * K, cast to bf16, gather all shards
                    for lt, nl in enumerate(lt_sizes):
                        l0 = lt * P
                        tb = small.tile([P, KSH], BF16, tag="tb")
                        nc.vector.tensor_scalar_mul(tb[:nl, :], Q[lt][:nl, :], float(K))
                        nc.sync.dma_start(t_shard[l0:l0 + nl, :], tb[:nl, :])
                    nc.gpsimd.collective_compute(
                        "AllGather", ALU.bypass, replica_groups=groups,
                        ins=[t_shard.opt()], outs=[t_all.opt()])

                if stop_after == "full":
                    # =========================================================
                    # Phase 4: masked CE over student tiles
                    # =========================================================
                    t_sb = []
                    for lt, nl in enumerate(lt_sizes):
                        l0 = lt * P
                        ts = tpool.tile([P, K], BF16, tag="t_sb")
                        nc.sync.dma_start(
                            ts[:nl, :].rearrange("l (c k) -> l c k", c=C),
                            t_all[:, l0:l0 + nl, :].rearrange("c l k -> l c k"))
                        t_sb.append(ts)

                    inv_t = 1.0 / STUDENT_TEMP
                    for b in range(B_loc):
                        for lt, nl in enumerate(lt_sizes):
                            j = b * n_lt + lt
                            l0 = lt * P
                            s = spool.tile([P, K], BF16, tag="s")
                            nc.sync.dma_start(s[:nl, :], student[b, l0:l0 + nl, :])

                            # exp((x/T) - SHIFT): keeps Z inside ScalarE Ln's domain
                            # (Ln input must be < 2^64; raw Z can reach ~1e23).
                            z = small.tile([P, 1], F32, tag="z")
                            escr = scrp.tile([P, K], BF16, tag="escr",
                                             name=f"escr{j}", bufs=2)
                            nc.scalar.activation(escr[:nl, :], s[:nl, :], AF.Exp,
                                                 scale=inv_t, bias=nshift[:nl, 0:1])
                            _tree_reduce_sum(
                                nc, lambda lo, hi: escr[:nl, lo:hi], K,
                                z[:nl, 0:1])
                            d = small.tile([P, 1], F32, tag="d")
                            mscr = scrp.tile([P, K], BF16, tag="mscr",
                                             name=f"mscr{j}")
                            nc.vector.tensor_mul(mscr[:nl, :], s[:nl, :],
                                                 t_sb[lt][:nl, :])
                            _tree_reduce_sum(
                                nc, lambda lo, hi: mscr[:nl, lo:hi], K,
                                d[:nl, 0:1])
                            lse = small.tile([P, 1], F32, tag="lse")
                            nc.scalar.activation(lse[:nl, 0:1], z[:nl, 0:1], AF.Ln)
                            pp = small.tile([P, 1], F32, tag="pp")
                            nc.vector.scalar_tensor_tensor(
                                pp[:nl, 0:1], in0=d[:nl, 0:1], scalar=-inv_t,
                                in1=lse[:nl, 0:1], op0=ALU.mult, op1=ALU.add)
                            nc.vector.tensor_mul(res[:nl, j:j + 1], pp[:nl, 0:1],
                                                 mask_sb[:nl, j:j + 1])

            acc = small.tile([P, 1], F32, tag="acc")
            nc.vector.reduce_sum(acc[:, 0:1], res[:, :], axis=AX.X)
            nc.sync.dma_start(out_ext[0:P, 0:1], acc[:, 0:1])
            nc.sync.dma_start(out_ext[P:2 * P, 0:1], rsq[:, 0:1])

    nc.compile()
    return nc


_NC_CACHE = {}


def _get_nc(key, builder):
    if key not in _NC_CACHE:
        _NC_CACHE[key] = builder()
    return _NC_CACHE[key]


def prepare_inputs(student_Q, teacher_Q, recon, label, prototype,
                   patches_labels, epoch, B_loc, L, K, C, R):
    """Host-side prep: dtype conversion, sharding, per-core in_maps."""
    KSH = K // C
    n_lt = _ceil_div(L, P)
    n_t = B_loc * n_lt
    epoch = int(np.asarray(epoch))

    student_Q = np.asarray(student_Q, dtype=np.float32)
    teacher_Q = np.asarray(teacher_Q, dtype=np.float32)
    recon = np.asarray(recon, dtype=np.float32)
    label = np.asarray(label, dtype=np.float32)
    prototype = np.asarray(prototype, dtype=np.float32)
    patches_labels = np.asarray(patches_labels)

    bf = ml_dtypes.bfloat16
    s_bf = student_Q.astype(bf)
    t_bf = teacher_Q.astype(bf)
    r_bf = recon.reshape(C, P, R).astype(bf)
    lb_bf = label.reshape(C, P, R).astype(bf)

    if epoch == 0:
        pscale, iscale = 0.0, 1.0 / (C * B_loc * SK_EPS)
    else:
        pscale = PROTO_MOMENTUM / SK_EPS
        iscale = (1.0 - PROTO_MOMENTUM) / (C * B_loc * SK_EPS)

    proto_s = (prototype[0] * pscale).astype(np.float32)        # [L, K]
    mask_full = (patches_labels == 0).astype(np.float32)        # [B, L]

    cfg_arr = np.full((P, 1), iscale, dtype=np.float32)

    in_maps = []
    for c in range(C):
        b0 = c * B_loc
        m = np.zeros((P, n_t), dtype=np.float32)
        for b in range(B_loc):
            for lt in range(n_lt):
                nl = min(P, L - lt * P)
                m[:nl, b * n_lt + lt] = mask_full[b0 + b, lt * P:lt * P + nl]
        in_maps.append({
            "student": np.ascontiguousarray(s_bf[b0:b0 + B_loc]),
            "teacher": np.ascontiguousarray(t_bf[b0:b0 + B_loc]),
            "recon": np.ascontiguousarray(r_bf[c]),
            "label": np.ascontiguousarray(lb_bf[c]),
            "proto": np.ascontiguousarray(proto_s[:, c * KSH:(c + 1) * KSH]),
            "maskp": m,
            "cfg": cfg_arr,
        })
    mask_cnt = float(mask_full.sum())
    return in_maps, mask_cnt


def finalize(results, mask_cnt, recon_size, B_loc=8, L=196):
    cst_num = 0.0
    rsq = 0.0
    for r in results:
        o = np.asarray(r["out"], dtype=np.float64).reshape(-1)
        cst_num += o[:P].sum()
        rsq += o[P:].sum()
    loss = cst_num / mask_cnt + LSE_SHIFT + rsq / recon_size
    return np.asarray(loss, dtype=np.float32).reshape(())


def kernel(student_Q, teacher_Q, recon, label, prototype, patches_labels,
           epoch, _trace=False):
    B, L, K = 64, 196, 8192
    C = N_CORES
    B_loc = B // C
    R = B_loc * 3 * 224 * 224 // P
    nc = _get_nc(("full",), lambda: build_nc(B_loc, L, K, C, R))
    in_maps, mask_cnt = prepare_inputs(
        student_Q, teacher_Q, recon, label, prototype, patches_labels, epoch,
        B_loc, L, K, C, R)
    res = run_bass_kernel_spmd(nc, in_maps, list(range(C)), trace=_trace)
    out = finalize(res.results, mask_cnt, float(np.asarray(recon).size), B_loc, L)
    if _trace:
        return out, res
    return out

